# revision 5
# baseline (speedup 1.0000x reference)
"""Trainium2 Bass kernel for DifferentiableCensus (3x3 soft census transform).

Full input x: [16, 3, 512, 512] f32. Output: same shape,
out = mean_{3x3 window, replicate pad} sigmoid(neighbor - center).

Sharding: pure data-parallel over batch: 8 cores x 2 batches; each core
processes 6 independent 512x512 images.

Algorithm (per image): symmetric-pair trick. With D4 = {E,S,SE,SW} and
t_d = sigmoid(shift_d(x) - x) (replicate-clamped), sigmoid(-u) = 1-sigmoid(u)
gives

  9*out[r,c] = 4.5 + sum_d t_d[r,c] - sum_d t_d[r-dr, c-dc]

exact everywhere provided t_d is evaluated on the replicate-extended grid
(clamped reads make every halo value correct by construction). Only 4
sigmoid maps instead of 9.

Layout: overlapped strip layout. Each SBUF tensor holds 2 images on 64
partitions each. Partition p of an image covers rows 8p-1 .. 8p+8 (8 data
rows + 1 halo row each side) as free-dim "slots", cols -1..512 as padded
columns. All stencil shifts are then pure free-dim offsets.

Engine distribution:
  - DMA (Pool SWDGE): ONE casting DMA per pair loads the 8 data rows per
    partition as fp16 straight from f32 HBM (software DGE casts in-flight,
    so no on-chip conversion pass and no f32 SBUF tile at all)
  - DMA (HWDGE/Sync): two SBUF->SBUF partition-shifted copies build the row
    halos on-chip (no duplicate HBM reads) + 2 stride-64 edge replicate
    fixes + 1 merged output store per half-pair
  - GpSimd:        column halos
  - Vector (DVE):  the 4 stencil diffs (fp16, 2x mode) + final affine
  - Scalar (ACT):  the 4 sigmoid maps
  - Tensor (PE):   the 8-term combine as +-identity matmuls accumulating
                   into PSUM (out9 = F - R), one 512-col slot per matmul

t maps are triple-buffered (one buffer per pair) so pair k+1's sigmoid/diff
stage never waits on pair k's matmul consumers.
"""

import numpy as np

import concourse.bacc as bacc
import concourse.mybir as mybir
import concourse.tile as tile
from concourse import masks
from concourse.bass_utils import run_bass_kernel_spmd

F16 = mybir.dt.float16
F32 = mybir.dt.float32
SIG = mybir.ActivationFunctionType.Sigmoid
MUL = mybir.AluOpType.mult
ADD = mybir.AluOpType.add

N_CORES = 8
NIMG = 6
H = W = 512
S = 8          # data rows per partition
NS = S + 2     # slots incl. top/bottom halo rows
WP = W + 2     # idx m = image col m-1 (cols -1..512)
PPI = H // S   # partitions per image = 64
IPT = 128 // PPI  # images per tensor = 2
NT = NIMG // IPT  # tensors per core = 3
NR = NS - 1    # 9 stencil rows per partition (rows 8p-1 .. 8p+7)
HB = 4         # psum slots per half-pair (4 banks)


def _emit_pair(nc, pools, wp, wn, xpair, ypair):
    """Emit ops for IPT images sharing one 128-partition tensor set.

    xpair/ypair: DRAM views [2*H, W] of this pair's two images (contiguous).
    """
    pmid, pt, ptmp, pps, pout = pools

    xh = pmid.tile([128, NS, WP], F16, name="xh")

    # main slots 1..8 = rows 8p..8p+7 of the flattened 1024-row pair,
    # cast f32 -> fp16 in-flight (software DGE on Pool).
    xs = xpair.rearrange("(p s) c -> p s c", s=S)  # [128, 8, 512]
    nc.gpsimd.dma_start(out=xh[:, 1 : S + 1, 1 : W + 1], in_=xs)

    # row halos on-chip (partition-shifted SBUF->SBUF copies):
    # slot 0[p] = row 8p-1 = slot 8[p-1];  slot 9[p] = row 8p+8 = slot 1[p+1]
    nc.sync.dma_start(
        out=xh[1:128, 0:1, 1 : W + 1], in_=xh[0:127, S : S + 1, 1 : W + 1]
    )
    nc.sync.dma_start(
        out=xh[0:127, S + 1 : S + 2, 1 : W + 1], in_=xh[1:128, 1:2, 1 : W + 1]
    )
    # replicate fixes at image edges, one stride-64 DMA each:
    # top rows (p=0,64: slot0 := row 0 = slot1)
    nc.sync.dma_start(
        out=xh[0:128:PPI, 0:1, 1 : W + 1], in_=xh[0:128:PPI, 1:2, 1 : W + 1]
    )
    # bottom rows (p=63,127: slot9 := row 511 = slot8)
    nc.sync.dma_start(
        out=xh[PPI - 1 : 128 : PPI, S + 1 : S + 2, 1 : W + 1],
        in_=xh[PPI - 1 : 128 : PPI, S : S + 1, 1 : W + 1],
    )
    # column halos: idx 0 := col 0 (idx 1), idx 513 := col 511 (idx 512)
    nc.gpsimd.tensor_copy(out=xh[:, :, 0:1], in_=xh[:, :, 1:2])
    nc.gpsimd.tensor_copy(out=xh[:, :, W + 1 : W + 2], in_=xh[:, :, W : W + 1])

    # ---- diffs (DVE) + sigmoids (ACT) on the extended grid ----
    # t_d[row, col] for slots 0..8 (rows -1..511); E only needs slots 1..8.
    tE = pt.tile([128, NR, WP], F16, name="tE")
    tS = pt.tile([128, NR, WP], F16, name="tS")
    tSE = pt.tile([128, NR, WP], F16, name="tSE")
    tSW = pt.tile([128, NR, WP], F16, name="tSW")

    # E: slots 1..8, idx 0..512 (cols -1..511): d = x[r, m+1] - x[r, m]
    dE = ptmp.tile([128, NR, WP], F16, name="dE", tag="d")
    nc.vector.tensor_sub(
        out=dE[:, 1:NR, 0 : W + 1],
        in0=xh[:, 1:NR, 1 : W + 2],
        in1=xh[:, 1:NR, 0 : W + 1],
    )
    nc.scalar.activation(
        out=tE[:, 1:NR, 0 : W + 1], in_=dE[:, 1:NR, 0 : W + 1], func=SIG
    )

    # S: slots 0..8, idx 1..512 (cols 0..511): d = x[r+1, m] - x[r, m]
    dS = ptmp.tile([128, NR, WP], F16, name="dS", tag="d")
    nc.vector.tensor_sub(
        out=dS[:, :, 1 : W + 1], in0=xh[:, 1:NS, 1 : W + 1], in1=xh[:, 0:NR, 1 : W + 1]
    )
    nc.scalar.activation(out=tS[:, :, 1 : W + 1], in_=dS[:, :, 1 : W + 1], func=SIG)

    # SE: slots 0..8, idx 0..512 (cols -1..511): d = x[r+1, m+1] - x[r, m]
    dSE = ptmp.tile([128, NR, WP], F16, name="dSE", tag="d")
    nc.vector.tensor_sub(
        out=dSE[:, :, 0 : W + 1],
        in0=xh[:, 1:NS, 1 : W + 2],
        in1=xh[:, 0:NR, 0 : W + 1],
    )
    nc.scalar.activation(out=tSE[:, :, 0 : W + 1], in_=dSE[:, :, 0 : W + 1], func=SIG)

    # SW: slots 0..8, idx 1..513 (cols 0..512): d = x[r+1, m-1] - x[r, m]
    dSW = ptmp.tile([128, NR, WP], F16, name="dSW", tag="d")
    nc.vector.tensor_sub(
        out=dSW[:, :, 1 : W + 2],
        in0=xh[:, 1:NS, 0 : W + 1],
        in1=xh[:, 0:NR, 1 : W + 2],
    )
    nc.scalar.activation(out=tSW[:, :, 1 : W + 2], in_=dSW[:, :, 1 : W + 2], func=SIG)

    # ---- combine on the Tensor engine ----
    # For output row r = 8p+s (s=0..7), image col c (t-map col idx c+1):
    #   forward (+1): all t_d at [slot s+1, idx c+1]
    #   reverse (-1): E [s+1, c], S [s, c+1], SE [s, c], SW [s, c+2]
    # PSUM accumulates F - R via identity weights (wp=+I, wn=-I), one
    # 512-col matmul per term per output slot; 4 slots per PSUM tile.
    ypr = ypair.rearrange("(p s) c -> p s c", s=S)  # [128, 8, 512]
    for half in range(2):
        ps = pps.tile([128, HB, W], F32, name="ps")
        for si in range(HB):
            s = half * HB + si
            bank = ps[:, si, :]
            terms = (
                (wp, tE[:, s + 1, 1 : W + 1]),
                (wp, tS[:, s + 1, 1 : W + 1]),
                (wp, tSE[:, s + 1, 1 : W + 1]),
                (wp, tSW[:, s + 1, 1 : W + 1]),
                (wn, tE[:, s + 1, 0:W]),
                (wn, tS[:, s, 1 : W + 1]),
                (wn, tSE[:, s, 0:W]),
                (wn, tSW[:, s, 2 : W + 2]),
            )
            for j, (w, rhs) in enumerate(terms):
                nc.tensor.matmul(
                    out=bank, lhsT=w, rhs=rhs, start=(j == 0), stop=(j == 7)
                )
        # out = (F - R)/9 + (4.5/9 = 0.5)
        of32 = pout.tile([128, HB, W], F32, name="of32")
        nc.vector.tensor_scalar(
            out=of32[:], in0=ps[:], scalar1=1.0 / 9.0, scalar2=0.5, op0=MUL, op1=ADD
        )
        # both images in one store: partition p -> pair rows 8p+4*half..+3
        nc.sync.dma_start(
            out=ypr[:, half * HB : (half + 1) * HB, :], in_=of32[:]
        )


_CACHED_NC = None


def _build():
    global _CACHED_NC
    if _CACHED_NC is not None:
        return _CACHED_NC
    nc = bacc.Bacc("TRN2", target_bir_lowering=False, debug=False)
    x = nc.dram_tensor("x", [NIMG, H, W], F32, kind="ExternalInput")
    y = nc.dram_tensor("y", [NIMG, H, W], F32, kind="ExternalOutput")
    xflat = x.ap().rearrange("i h c -> (i h) c")  # [3072, 512]
    yflat = y.ap().rearrange("i h c -> (i h) c")
    with tile.TileContext(nc) as tc:
        with (
            tc.tile_pool(name="pw", bufs=1) as pw,
            tc.tile_pool(name="pmid", bufs=2) as pmid,
            tc.tile_pool(name="pt", bufs=3) as pt,
            tc.tile_pool(name="ptmp", bufs=4) as ptmp,
            tc.psum_pool(name="pps", bufs=2) as pps,
            tc.tile_pool(name="pout", bufs=2) as pout,
        ):
            wp = pw.tile([128, 128], F16, name="wp")
            wn = pw.tile([128, 128], F16, name="wn")
            masks.make_identity(nc, wp[:])
            nc.gpsimd.memset(wn[:], 0.0)
            nc.gpsimd.affine_select(
                out=wn[:],
                in_=wn[:],
                compare_op=mybir.AluOpType.not_equal,
                fill=-1.0,
                base=0,
                pattern=[[-1, 128]],
                channel_multiplier=1,
            )
            pools = (pmid, pt, ptmp, pps, pout)
            for t in range(NT):
                _emit_pair(
                    nc,
                    pools,
                    wp[:],
                    wn[:],
                    xflat[t * IPT * H : (t + 1) * IPT * H, :],
                    yflat[t * IPT * H : (t + 1) * IPT * H, :],
                )
    nc.compile()
    _CACHED_NC = nc
    return nc


def kernel(x: np.ndarray) -> np.ndarray:
    assert x.shape == (16, 3, 512, 512) and x.dtype == np.float32
    nc = _build()
    xs = x.reshape(N_CORES, NIMG, H, W)
    in_maps = [{"x": np.ascontiguousarray(xs[i])} for i in range(N_CORES)]
    res = run_bass_kernel_spmd(nc, in_maps, core_ids=list(range(N_CORES)))
    out = np.stack([res.results[i]["y"] for i in range(N_CORES)])
    return out.reshape(16, 3, 512, 512)
